# revision 9
# baseline (speedup 1.0000x reference)
"""PointPillarScatter3d (scatter-mean into BEV grid) on 8 trn2 NeuronCores.

Strategy
--------
Shard pillars by (batch, y-half): core k owns sample b = k//2, rows
y in [h*180, (h+1)*180) with h = k%2.  Each core's output slab is
[C=128, 180*360=64800] and the scatter is purely local.

Per core the 64800 cells are cut into 270 "superblocks" of 240 cells.
The host buckets each pillar into its superblock (pure permutation +
padding; max observed fill is ~113 <= 128 slots).  The device then, per
superblock:
  onehot[p, j] = (cell[p] == j) * (1/count[cell[p]])     (one DVE op)
  psum[f, j]   = sum_p feat[p, f] * onehot[p, j]         (one matmul)
which is exactly the scatter-mean for those 240 columns.  PSUM is copied
to an SBUF staging buffer on the scalar engine and DMA'd out in 1.2 MB
blocks.  All feature arithmetic happens on device; the host only builds
index/permutation metadata (and integer counts, sent as f32).
"""

import numpy as np

B = 4
C = 128
NXY = 360
N = 160000
HALF_Y = 180
S_SH = HALF_Y * NXY          # 64800 cells per core
SB = 240                     # cells per superblock
NSB = S_SH // SB             # 270 superblocks per core
CAP = 128                    # pillar slots per superblock (max fill ~113)
CHUNK = 30                   # superblocks per input DMA chunk
NCHUNK = NSB // CHUNK        # 9
STAGE = 18                   # superblocks per output staging flush
SENT = 999.0                 # sentinel cell id for padding slots (never matches)

_cache = {}
LAST_RESULTS = None


def _build():
    from contextlib import ExitStack

    import concourse.bacc as bacc
    import concourse.mybir as mybir
    import concourse.tile as tile

    f32 = mybir.dt.float32
    f32r = mybir.dt.float32r
    i32 = mybir.dt.int32

    nc = bacc.Bacc("TRN2", target_bir_lowering=False, debug=False, num_devices=8)
    feats = nc.dram_tensor("feats", [CAP, NSB * C], f32r, kind="ExternalInput").ap()
    cellf = nc.dram_tensor("cellf", [CAP, NSB], f32, kind="ExternalInput").ap()
    cntf = nc.dram_tensor("cntf", [CAP, NSB], f32, kind="ExternalInput").ap()
    out = nc.dram_tensor("out", [C, S_SH], f32, kind="ExternalOutput").ap()

    with tile.TileContext(nc) as tc:
        with ExitStack() as ctx:
            oh_pool = ctx.enter_context(tc.tile_pool(name="oh", bufs=8))
            ps_pool = ctx.enter_context(tc.tile_pool(name="ps", bufs=8, space="PSUM"))
            pp = ctx.enter_context(tc.tile_pool(name="persist", bufs=1))

            iota_i = pp.tile([CAP, SB], i32, tag="iota_i", name="iota_i")
            iota_f = pp.tile([CAP, SB], f32, tag="iota_f", name="iota_f")
            nc.gpsimd.iota(iota_i[:], pattern=[[1, SB]], base=0, channel_multiplier=0)
            nc.vector.tensor_copy(iota_f[:], iota_i[:])

            cellf_t = pp.tile([CAP, NSB], f32, tag="cellf_t", name="cellf_t")
            cntf_t = pp.tile([CAP, NSB], f32, tag="cntf_t", name="cntf_t")
            recip_t = pp.tile([CAP, NSB], f32, tag="recip_t", name="recip_t")
            nc.sync.dma_start(out=cellf_t[:], in_=cellf[:, :])
            nc.sync.dma_start(out=cntf_t[:], in_=cntf[:, :])
            nc.vector.reciprocal(recip_t[:], cntf_t[:])

            gbufs = [
                pp.tile([CAP, CHUNK * C], f32r, tag=f"gb{i}", name=f"gb{i}")
                for i in range(4)
            ]
            stages = [
                pp.tile([C, STAGE * SB], f32, tag=f"st{i}", name=f"st{i}")
                for i in range(2)
            ]

            for ch in range(NCHUNK):
                g = gbufs[ch % 4]
                nc.sync.dma_start(
                    out=g[:], in_=feats[:, ch * CHUNK * C : (ch + 1) * CHUNK * C]
                )
                for tl in range(0, CHUNK, 2):
                    sb = ch * CHUNK + tl
                    # one PSUM bank holds two superblocks' outputs -> one
                    # ACT copy and one staging write per pair
                    ps = ps_pool.tile([C, 2 * SB], f32, tag="ps", name=f"ps{sb}")
                    for half in range(2):
                        oh = oh_pool.tile(
                            [CAP, SB], f32r, tag="oh", name=f"oh{sb + half}"
                        )
                        nc.vector.tensor_scalar(
                            out=oh[:],
                            in0=iota_f[:],
                            scalar1=cellf_t[:, sb + half : sb + half + 1],
                            scalar2=recip_t[:, sb + half : sb + half + 1],
                            op0=mybir.AluOpType.is_equal,
                            op1=mybir.AluOpType.mult,
                        )
                        nc.tensor.matmul(
                            out=ps[:, half * SB : (half + 1) * SB],
                            lhsT=g[:, (tl + half) * C : (tl + half + 1) * C],
                            rhs=oh[:],
                            start=True,
                            stop=True,
                        )
                    st = stages[(sb // STAGE) % 2]
                    nc.scalar.copy(
                        out=st[:, (sb % STAGE) * SB : (sb % STAGE + 2) * SB],
                        in_=ps[:],
                    )
                    if sb % STAGE == STAGE - 2:
                        grp = sb // STAGE
                        nc.scalar.dma_start(
                            out=out[:, grp * STAGE * SB : (grp + 1) * STAGE * SB],
                            in_=st[:],
                        )
    nc.compile()
    return nc


def _prep(pf, vc):
    """Host-side sharding: permutation/bucketing + padding only (no feature math)."""
    pf = np.ascontiguousarray(pf, dtype=np.float32)
    vc = np.asarray(vc)
    in_maps = []
    for k in range(8):
        b, h = k // 2, k % 2
        y0 = h * HALF_Y
        m = (vc[:, 0] == b) & (vc[:, 2] >= y0) & (vc[:, 2] < y0 + HALF_Y)
        idx = np.nonzero(m)[0]
        cell = (vc[idx, 2].astype(np.int64) - y0) * NXY + vc[idx, 3]
        counts = np.bincount(cell, minlength=S_SH)
        order = np.argsort(cell, kind="stable")
        idx, cell = idx[order], cell[order]
        sbid = cell // SB
        fill = np.bincount(sbid, minlength=NSB)
        assert fill.max() <= CAP, f"superblock overflow: {fill.max()} > {CAP}"
        starts = np.concatenate([[0], np.cumsum(fill)[:-1]])
        slot = np.arange(len(idx)) - starts[sbid]

        feats = np.zeros((CAP, NSB, C), np.float32)
        feats[slot, sbid, :] = pf[idx]
        cellf = np.full((CAP, NSB), SENT, np.float32)
        cellf[slot, sbid] = (cell % SB).astype(np.float32)
        cntf = np.ones((CAP, NSB), np.float32)
        cntf[slot, sbid] = counts[cell].astype(np.float32)
        in_maps.append(
            {
                "feats": np.ascontiguousarray(feats.reshape(CAP, NSB * C)),
                "cellf": cellf,
                "cntf": cntf,
            }
        )
    return in_maps


def kernel(pillar_features, voxel_coords):
    global LAST_RESULTS
    from concourse import bass_utils

    if "nc" not in _cache:
        _cache["nc"] = _build()
    nc = _cache["nc"]
    in_maps = _prep(np.asarray(pillar_features), np.asarray(voxel_coords))
    res = bass_utils.run_bass_kernel_spmd(nc, in_maps, core_ids=list(range(8)))
    LAST_RESULTS = res
    out = np.zeros((B, C, NXY, NXY), np.float32)
    for k in range(8):
        b, h = k // 2, k % 2
        out[b, :, h * HALF_Y : (h + 1) * HALF_Y, :] = res.results[k]["out"].reshape(
            C, HALF_Y, NXY
        )
    return out


# revision 12
# speedup vs baseline: 1.2160x; 1.2160x over previous
"""PointPillarScatter3d (scatter-mean into BEV grid) on 8 trn2 NeuronCores.

Strategy
--------
Shard pillars by (batch, y-half): core k owns sample b = k//2, rows
y in [h*180, (h+1)*180) with h = k%2.  Each core's output slab is
[C=128, 180*360=64800] and the scatter is purely local.

Per core the 64800 cells are cut into 270 "superblocks" of 240 cells.
The host buckets each pillar into its superblock (pure permutation +
padding; max observed fill is ~113 <= 128 slots).  The device then, per
superblock:
  onehot[p, j] = (cell[p] == j) * (1/count[cell[p]])     (one DVE op)
  psum[f, j]   = sum_p feat[p, f] * onehot[p, j]         (one matmul)
which is exactly the scatter-mean for those 240 columns.  PSUM is copied
to an SBUF staging buffer on the scalar engine and DMA'd out in 1.2 MB
blocks.  All feature arithmetic happens on device; the host only builds
index/permutation metadata (and integer counts, sent as f32).
"""

import numpy as np

B = 4
C = 128
NXY = 360
N = 160000
HALF_Y = 180
S_SH = HALF_Y * NXY          # 64800 cells per core
SB = 240                     # cells per superblock
NSB = S_SH // SB             # 270 superblocks per core
CAP = 128                    # pillar slots per superblock (max fill ~113)
CHUNK = 30                   # superblocks per input DMA chunk
NCHUNK = NSB // CHUNK        # 9
STAGE = 18                   # superblocks per output staging flush
SENT = 999.0                 # sentinel cell id for padding slots (never matches)

_cache = {}
LAST_RESULTS = None


def _build():
    from contextlib import ExitStack

    import concourse.bacc as bacc
    import concourse.mybir as mybir
    import concourse.tile as tile

    f32 = mybir.dt.float32
    f16 = mybir.dt.float16
    i32 = mybir.dt.int32

    nc = bacc.Bacc("TRN2", target_bir_lowering=False, debug=False, num_devices=8)
    feats = nc.dram_tensor("feats", [CAP, NSB * C], f16, kind="ExternalInput").ap()
    cellf = nc.dram_tensor("cellf", [CAP, NSB], f32, kind="ExternalInput").ap()
    cntf = nc.dram_tensor("cntf", [CAP, NSB], f32, kind="ExternalInput").ap()
    out = nc.dram_tensor("out", [C, S_SH], f32, kind="ExternalOutput").ap()

    with tile.TileContext(nc) as tc:
        with ExitStack() as ctx:
            oh_pool = ctx.enter_context(tc.tile_pool(name="oh", bufs=8))
            ps_pool = ctx.enter_context(tc.tile_pool(name="ps", bufs=8, space="PSUM"))
            pp = ctx.enter_context(tc.tile_pool(name="persist", bufs=1))

            iota_i = pp.tile([CAP, SB], i32, tag="iota_i", name="iota_i")
            iota_f = pp.tile([CAP, SB], f16, tag="iota_f", name="iota_f")
            nc.gpsimd.iota(iota_i[:], pattern=[[1, SB]], base=0, channel_multiplier=0)
            nc.vector.tensor_copy(iota_f[:], iota_i[:])

            cellf_t = pp.tile([CAP, NSB], f32, tag="cellf_t", name="cellf_t")
            cntf_t = pp.tile([CAP, NSB], f32, tag="cntf_t", name="cntf_t")
            recip_t = pp.tile([CAP, NSB], f32, tag="recip_t", name="recip_t")
            nc.sync.dma_start(out=cellf_t[:], in_=cellf[:, :])
            nc.sync.dma_start(out=cntf_t[:], in_=cntf[:, :])
            nc.vector.reciprocal(recip_t[:], cntf_t[:])

            gbufs = [
                pp.tile([CAP, CHUNK * C], f16, tag=f"gb{i}", name=f"gb{i}")
                for i in range(4)
            ]
            stages = [
                pp.tile([C, STAGE * SB], f32, tag=f"st{i}", name=f"st{i}")
                for i in range(2)
            ]

            for ch in range(NCHUNK):
                g = gbufs[ch % 4]
                nc.sync.dma_start(
                    out=g[:], in_=feats[:, ch * CHUNK * C : (ch + 1) * CHUNK * C]
                )
                for tl in range(0, CHUNK, 2):
                    sb = ch * CHUNK + tl
                    # one PSUM bank holds two superblocks' outputs -> one
                    # ACT copy and one staging write per pair
                    ps = ps_pool.tile([C, 2 * SB], f32, tag="ps", name=f"ps{sb}")
                    for half in range(2):
                        oh = oh_pool.tile(
                            [CAP, SB], f16, tag="oh", name=f"oh{sb + half}"
                        )
                        nc.vector.tensor_scalar(
                            out=oh[:],
                            in0=iota_f[:],
                            scalar1=cellf_t[:, sb + half : sb + half + 1],
                            scalar2=recip_t[:, sb + half : sb + half + 1],
                            op0=mybir.AluOpType.is_equal,
                            op1=mybir.AluOpType.mult,
                        )
                        nc.tensor.matmul(
                            out=ps[:, half * SB : (half + 1) * SB],
                            lhsT=g[:, (tl + half) * C : (tl + half + 1) * C],
                            rhs=oh[:],
                            start=True,
                            stop=True,
                        )
                    st = stages[(sb // STAGE) % 2]
                    nc.scalar.copy(
                        out=st[:, (sb % STAGE) * SB : (sb % STAGE + 2) * SB],
                        in_=ps[:],
                    )
                    if sb % STAGE == STAGE - 2:
                        grp = sb // STAGE
                        nc.scalar.dma_start(
                            out=out[:, grp * STAGE * SB : (grp + 1) * STAGE * SB],
                            in_=st[:],
                        )
    nc.compile()
    return nc


def _prep(pf, vc):
    """Host-side sharding: permutation/bucketing + padding only (no feature math)."""
    pf = np.ascontiguousarray(pf, dtype=np.float32)
    vc = np.asarray(vc)
    in_maps = []
    for k in range(8):
        b, h = k // 2, k % 2
        y0 = h * HALF_Y
        m = (vc[:, 0] == b) & (vc[:, 2] >= y0) & (vc[:, 2] < y0 + HALF_Y)
        idx = np.nonzero(m)[0]
        cell = (vc[idx, 2].astype(np.int64) - y0) * NXY + vc[idx, 3]
        counts = np.bincount(cell, minlength=S_SH)
        order = np.argsort(cell, kind="stable")
        idx, cell = idx[order], cell[order]
        sbid = cell // SB
        fill = np.bincount(sbid, minlength=NSB)
        assert fill.max() <= CAP, f"superblock overflow: {fill.max()} > {CAP}"
        starts = np.concatenate([[0], np.cumsum(fill)[:-1]])
        slot = np.arange(len(idx)) - starts[sbid]

        feats = np.zeros((CAP, NSB, C), np.float32)
        feats[slot, sbid, :] = pf[idx]
        cellf = np.full((CAP, NSB), SENT, np.float32)
        cellf[slot, sbid] = (cell % SB).astype(np.float32)
        cntf = np.ones((CAP, NSB), np.float32)
        cntf[slot, sbid] = counts[cell].astype(np.float32)
        in_maps.append(
            {
                "feats": np.ascontiguousarray(
                    feats.reshape(CAP, NSB * C).astype(np.float16)
                ),
                "cellf": cellf,
                "cntf": cntf,
            }
        )
    return in_maps


def kernel(pillar_features, voxel_coords):
    global LAST_RESULTS
    from concourse import bass_utils

    if "nc" not in _cache:
        _cache["nc"] = _build()
    nc = _cache["nc"]
    in_maps = _prep(np.asarray(pillar_features), np.asarray(voxel_coords))
    res = bass_utils.run_bass_kernel_spmd(nc, in_maps, core_ids=list(range(8)))
    LAST_RESULTS = res
    out = np.zeros((B, C, NXY, NXY), np.float32)
    for k in range(8):
        b, h = k // 2, k % 2
        out[b, :, h * HALF_Y : (h + 1) * HALF_Y, :] = res.results[k]["out"].reshape(
            C, HALF_Y, NXY
        )
    return out


# revision 14
# speedup vs baseline: 1.3204x; 1.0858x over previous
"""PointPillarScatter3d (scatter-mean into BEV grid) on 8 trn2 NeuronCores.

Strategy
--------
Shard pillars by (batch, y-half): core k owns sample b = k//2, rows
y in [h*180, (h+1)*180) with h = k%2.  Each core's output slab is
[C=128, 180*360=64800] and the scatter is purely local.

Per core the 64800 cells are cut into 270 "superblocks" of 240 cells.
The host buckets each pillar into its superblock (pure permutation +
padding; max observed fill is ~113 <= 128 slots).  The device then, per
superblock:
  onehot[p, j] = (cell[p] == j) * (1/count[cell[p]])     (one DVE op)
  psum[f, j]   = sum_p feat[p, f] * onehot[p, j]         (one matmul)
which is exactly the scatter-mean for those 240 columns.  PSUM is copied
to an SBUF staging buffer on the scalar engine and DMA'd out in 1.2 MB
blocks.  All feature arithmetic happens on device; the host only builds
index/permutation metadata (and integer counts, sent as f32).
"""

import numpy as np

B = 4
C = 128
NXY = 360
N = 160000
HALF_Y = 180
S_SH = HALF_Y * NXY          # 64800 cells per core
SB = 240                     # cells per superblock
NSB = S_SH // SB             # 270 superblocks per core
CAP = 128                    # pillar slots per superblock (max fill ~113)
CHUNK = 54                   # superblocks per input DMA chunk
NCHUNK = NSB // CHUNK        # 5
STAGE = 18                   # superblocks per output staging flush
SENT = 999.0                 # sentinel cell id for padding slots (never matches)

_cache = {}
LAST_RESULTS = None


def _build():
    from contextlib import ExitStack

    import concourse.bacc as bacc
    import concourse.mybir as mybir
    import concourse.tile as tile

    f32 = mybir.dt.float32
    f16 = mybir.dt.float16
    i32 = mybir.dt.int32

    nc = bacc.Bacc("TRN2", target_bir_lowering=False, debug=False, num_devices=8)
    feats = nc.dram_tensor("feats", [CAP, NSB * C], f16, kind="ExternalInput").ap()
    cellf = nc.dram_tensor("cellf", [CAP, NSB], f32, kind="ExternalInput").ap()
    cntf = nc.dram_tensor("cntf", [CAP, NSB], f32, kind="ExternalInput").ap()
    out = nc.dram_tensor("out", [C, S_SH], f32, kind="ExternalOutput").ap()

    with tile.TileContext(nc) as tc:
        with ExitStack() as ctx:
            oh_pool = ctx.enter_context(tc.tile_pool(name="oh", bufs=8))
            ps_pool = ctx.enter_context(tc.tile_pool(name="ps", bufs=8, space="PSUM"))
            pp = ctx.enter_context(tc.tile_pool(name="persist", bufs=1))

            iota_i = pp.tile([CAP, SB], i32, tag="iota_i", name="iota_i")
            iota_f = pp.tile([CAP, SB], f16, tag="iota_f", name="iota_f")
            nc.gpsimd.iota(iota_i[:], pattern=[[1, SB]], base=0, channel_multiplier=0)
            nc.vector.tensor_copy(iota_f[:], iota_i[:])

            cellf_t = pp.tile([CAP, NSB], f32, tag="cellf_t", name="cellf_t")
            cntf_t = pp.tile([CAP, NSB], f32, tag="cntf_t", name="cntf_t")
            recip_t = pp.tile([CAP, NSB], f32, tag="recip_t", name="recip_t")
            nc.scalar.dma_start(out=cellf_t[:], in_=cellf[:, :])
            nc.scalar.dma_start(out=cntf_t[:], in_=cntf[:, :])
            nc.vector.reciprocal(recip_t[:], cntf_t[:])

            gbufs = [
                pp.tile([CAP, CHUNK * C], f16, tag=f"gb{i}", name=f"gb{i}")
                for i in range(4)
            ]
            stages = [
                pp.tile([C, STAGE * SB], f32, tag=f"st{i}", name=f"st{i}")
                for i in range(2)
            ]

            for ch in range(NCHUNK):
                g = gbufs[ch % 4]
                nc.sync.dma_start(
                    out=g[:], in_=feats[:, ch * CHUNK * C : (ch + 1) * CHUNK * C]
                )
                for tl in range(0, CHUNK, 2):
                    sb = ch * CHUNK + tl
                    # one PSUM bank holds two superblocks' outputs -> one
                    # ACT copy and one staging write per pair
                    ps = ps_pool.tile([C, 2 * SB], f32, tag="ps", name=f"ps{sb}")
                    for half in range(2):
                        oh = oh_pool.tile(
                            [CAP, SB], f16, tag="oh", name=f"oh{sb + half}"
                        )
                        nc.vector.tensor_scalar(
                            out=oh[:],
                            in0=iota_f[:],
                            scalar1=cellf_t[:, sb + half : sb + half + 1],
                            scalar2=recip_t[:, sb + half : sb + half + 1],
                            op0=mybir.AluOpType.is_equal,
                            op1=mybir.AluOpType.mult,
                        )
                        nc.tensor.matmul(
                            out=ps[:, half * SB : (half + 1) * SB],
                            lhsT=g[:, (tl + half) * C : (tl + half + 1) * C],
                            rhs=oh[:],
                            start=True,
                            stop=True,
                        )
                    st = stages[(sb // STAGE) % 2]
                    nc.scalar.copy(
                        out=st[:, (sb % STAGE) * SB : (sb % STAGE + 2) * SB],
                        in_=ps[:],
                    )
                    half_sbs = 10
                    if sb % STAGE == half_sbs - 2:
                        grp = sb // STAGE
                        nc.scalar.dma_start(
                            out=out[
                                :,
                                grp * STAGE * SB : (grp * STAGE + half_sbs) * SB,
                            ],
                            in_=st[:, : half_sbs * SB],
                        )
                    elif sb % STAGE == STAGE - 2:
                        grp = sb // STAGE
                        nc.scalar.dma_start(
                            out=out[
                                :,
                                (grp * STAGE + half_sbs) * SB : (grp + 1) * STAGE * SB,
                            ],
                            in_=st[:, half_sbs * SB :],
                        )
    nc.compile()
    return nc


def _prep(pf, vc):
    """Host-side sharding: permutation/bucketing + padding only (no feature math)."""
    pf = np.ascontiguousarray(pf, dtype=np.float32)
    vc = np.asarray(vc)
    in_maps = []
    for k in range(8):
        b, h = k // 2, k % 2
        y0 = h * HALF_Y
        m = (vc[:, 0] == b) & (vc[:, 2] >= y0) & (vc[:, 2] < y0 + HALF_Y)
        idx = np.nonzero(m)[0]
        cell = (vc[idx, 2].astype(np.int64) - y0) * NXY + vc[idx, 3]
        counts = np.bincount(cell, minlength=S_SH)
        order = np.argsort(cell, kind="stable")
        idx, cell = idx[order], cell[order]
        sbid = cell // SB
        fill = np.bincount(sbid, minlength=NSB)
        assert fill.max() <= CAP, f"superblock overflow: {fill.max()} > {CAP}"
        starts = np.concatenate([[0], np.cumsum(fill)[:-1]])
        slot = np.arange(len(idx)) - starts[sbid]

        feats = np.zeros((CAP, NSB, C), np.float32)
        feats[slot, sbid, :] = pf[idx]
        cellf = np.full((CAP, NSB), SENT, np.float32)
        cellf[slot, sbid] = (cell % SB).astype(np.float32)
        cntf = np.ones((CAP, NSB), np.float32)
        cntf[slot, sbid] = counts[cell].astype(np.float32)
        in_maps.append(
            {
                "feats": np.ascontiguousarray(
                    feats.reshape(CAP, NSB * C).astype(np.float16)
                ),
                "cellf": cellf,
                "cntf": cntf,
            }
        )
    return in_maps


def kernel(pillar_features, voxel_coords):
    global LAST_RESULTS
    from concourse import bass_utils

    if "nc" not in _cache:
        _cache["nc"] = _build()
    nc = _cache["nc"]
    in_maps = _prep(np.asarray(pillar_features), np.asarray(voxel_coords))
    res = bass_utils.run_bass_kernel_spmd(nc, in_maps, core_ids=list(range(8)))
    LAST_RESULTS = res
    out = np.zeros((B, C, NXY, NXY), np.float32)
    for k in range(8):
        b, h = k // 2, k % 2
        out[b, :, h * HALF_Y : (h + 1) * HALF_Y, :] = res.results[k]["out"].reshape(
            C, HALF_Y, NXY
        )
    return out
